# revision 1
# baseline (speedup 1.0000x reference)
"""RNN-T joint network (Conformer transducer) kernel for Trainium2.

Computes out[b,t,u,v] = (enc[b,t,:] @ W[:, :D].T)[v] + (dec[b,u,:] @ W[:, D:].T)[v]
i.e. the broadcast-sum decomposition of cat(enc, dec) @ W.T without
materialising the (B,T,U,2D) concat.

Sharding: the (B*T) = 1024 grid rows are split across 8 NeuronCores
(cores 0-3 take b=0, cores 4-7 take b=1, 128 t-rows each). W is
replicated. Each core emits its own 32 MB fp16 slab; the host
reassembles the full fp32 (B,T,U,V) tensor. fp16 output keeps the
max-relative error ~3e-3, well inside the 2e-2 budget, and halves the
HBM write traffic.

Per-core structure (all matmul operands bf16 - the PE runs bf16 at
1 cycle/column vs 2 for fp16):
  1. enc_proj / dec_proj bf16 matmuls on the TensorEngine (K=512 in 4
     chunks). Each K-chunk's lhsT and rhs live in one packed SBUF tile
     fed by a single DMA.
  2. Projections are rounded to bf16 (enc_hi / dec2; dec2 duplicated so
     one FD=1024 DVE add covers a pair of tiles). A dif4 matmul also
     produces enc_dif[t] = enc_hi[t] - enc_hi[t-4].
  3. Tiles are processed in pairs. Per t, a one-hot matmul broadcasts
     row t across all 128 PSUM partitions; the "selector" is column t
     of the identity matrix expanded by a stride-0 access pattern, so
     no selector tensor is ever loaded. Every main-loop matmul keeps
     the same (128,128) PE tile config: a config switch stalls the
     array, poisoning the HAM activity window and de-rating the PE
     clock from 2.4 to 1.2 GHz.
  4. v-lower halves go to a rotating PSUM tile; v-upper halves live in
     persistent PSUM "chains" that hold dec + enc(t) and accumulate
     enc_dif per step (re-seeded with dec via identity matmuls every 4
     groups to cap the bf16 rounding walk). This removes the per-tile
     dec matmul, keeping the PE off the critical path.
  5. The drain is split across two engines reading disjoint PSUM tiles
     (the Tile framework chains accessors of a shared tile, which
     would serialise them): VectorE computes fp16(ps_lo + dec2),
     ScalarE copies fp16(chain), FD=1024 per instruction, into
     separate SBUF tiles (again to avoid accessor chaining).
  6. Four t-tiles form one DMA group written by two 512 KB HWDGE DMAs
     (4 KB per-partition descriptors); the host un-permutes the group
     layout while upcasting to fp32.

The kernel is wire-limited: ~32 MB of fp16 output per core at the
~358 GB/s per-core HBM cap, plus ~2.4 MB of inputs.
"""

import numpy as np

import bass_rust
import concourse.bass as bass
import concourse.tile as tile
from concourse import bacc
from concourse import mybir
from concourse.bass_utils import run_bass_kernel_spmd

_vec_pair = bass_rust.VecI64Pair

B, T, U, D, V = 2, 512, 128, 512, 1024
N_CORES = 8
T_LOC = (B * T) // N_CORES  # 128 t-rows per core
PKW = 128 + V  # packed chunk width: [lhsT column block | rhs row block]
TG = 4  # t-tiles per output DMA group

F32 = mybir.dt.float32
F16 = mybir.dt.float16
BF16 = mybir.dt.bfloat16


def _build_program() -> bass.Bass:
    nc = bacc.Bacc("TRN2", debug=False, num_devices=N_CORES)

    # PACK[kc] = [encT chunk kc | WT chunk kc]        for kc in 0..3
    #          = [decT chunk kc-4 | WT chunk kc]      for kc in 4..7
    PACK = nc.dram_tensor("PACK", [8, 128, PKW], BF16, kind="ExternalInput").ap()
    IDNR = nc.dram_tensor("IDNR", [128, 128], BF16, kind="ExternalInput").ap()
    # DIFR[k, t] = dif4 matrix: enc_dif[t] = enc_hi[t] - enc_hi[t-4] (t >= 4)
    DIFR = nc.dram_tensor("DIFR", [128, 128], BF16, kind="ExternalInput").ap()
    # out[grp, half, u, tt, vl] = true_out[TG*grp + tt, u, 512*half + vl];
    # the v-halves are separate so the DVE and ACT drain into separate SBUF
    # tiles (two writers to one tile would serialise); host un-permutes.
    OUT = nc.dram_tensor(
        "out", [T_LOC // TG, 2, U, TG, 512], F16, kind="ExternalOutput"
    ).ap()

    with tile.TileContext(nc) as tc:
        with (
            tc.tile_pool(name="const", bufs=1) as cpool,
            tc.tile_pool(name="pmain", bufs=2, space="PSUM") as pmain,
            tc.tile_pool(name="pchain", bufs=1, space="PSUM") as pchain,
            tc.tile_pool(name="outp", bufs=12) as opool,
        ):
            # ---- inputs to SBUF ----
            # dec chunks (4-7) first: the dec projection runs first on the PE.
            pk = [None] * 8
            for kc in (4, 5, 6, 7, 0, 1, 2, 3):
                tl = cpool.tile([128, PKW], BF16, tag=f"pk{kc}")
                nc.sync.dma_start(out=tl[:], in_=PACK[kc])
                pk[kc] = tl
            idn = cpool.tile([128, 128], BF16, tag="idn")
            nc.sync.dma_start(out=idn[:], in_=IDNR)
            dif = cpool.tile([128, 128], BF16, tag="dif")
            nc.sync.dma_start(out=dif[:], in_=DIFR)

            # ---- dec_proj = decT.T @ W_decT : (U, V) ----
            dec_ps = pmain.tile([128, 2, 512], F32, tag="ps")
            for vh in range(2):
                for kc in range(4):
                    nc.tensor.matmul(
                        dec_ps[:, vh, :],
                        lhsT=pk[4 + kc][:, 0:128],
                        rhs=pk[4 + kc][:, 128 + 512 * vh : 128 + 512 * (vh + 1)],
                        start=(kc == 0),
                        stop=(kc == 3),
                    )
            # bf16 copy on the ScalarEngine: warms the ACT table and keeps
            # the DVE free; duplicate for pair-wide DVE adds.
            dec2 = cpool.tile([128, 2, V], BF16, tag="dec2")
            nc.scalar.copy(out=dec2[:, 0, :], in_=dec_ps[:])
            nc.vector.tensor_copy(out=dec2[:, 1, :], in_=dec2[:, 0, :])

            # ---- enc_proj = encT.T @ W_encT : (T_LOC, V) ----
            enc_ps = pmain.tile([128, 2, 512], F32, tag="ps")
            for vh in range(2):
                for kc in range(4):
                    nc.tensor.matmul(
                        enc_ps[:, vh, :],
                        lhsT=pk[kc][:, 0:128],
                        rhs=pk[kc][:, 128 + 512 * vh : 128 + 512 * (vh + 1)],
                        start=(kc == 0),
                        stop=(kc == 3),
                    )
            enc_hi = cpool.tile([128, V], BF16, tag="ehi")
            nc.vector.tensor_copy(out=enc_hi[:], in_=enc_ps[:])

            # enc_dif[t] = enc_hi[t] - enc_hi[t-4]: lets the v-upper PSUM
            # chains accumulate a diff per step instead of re-adding dec.
            dif_ps = pmain.tile([128, 2, 512], F32, tag="ps")
            for vh in range(2):
                nc.tensor.matmul(
                    dif_ps[:, vh, :],
                    lhsT=dif[:],
                    rhs=enc_hi[:, 512 * vh : 512 * (vh + 1)],
                    start=True,
                    stop=True,
                )
            enc_dif = cpool.tile([128, V], BF16, tag="edif")
            nc.vector.tensor_copy(out=enc_dif[:], in_=dif_ps[:])

            # Persistent v-upper PSUM chains: chain[p][:, tt, :] holds
            # dec + enc(t) for t = 4*i + 2*p + tt; each step accumulates
            # enc_dif via a one-hot matmul instead of re-adding dec.
            chain0 = pchain.tile([128, 2, 512], F32, tag="chain0")
            chain1 = pchain.tile([128, 2, 512], F32, tag="chain1")
            chains = [chain0, chain1]

            # ---- main loop: TG t-tiles per DMA group, pairs per PSUM tile ----
            for grp in range(T_LOC // TG):
                ob_dve = opool.tile([128, TG, 512], F16, tag="obd")
                ob_act = opool.tile([128, TG, 512], F16, tag="oba")
                # Re-seed the chains every 4 groups to cap the accumulated
                # bf16 rounding walk of the dif steps.
                init = grp % 4 == 0
                for half in range(TG // 2):
                    t0 = TG * grp + 2 * half
                    # v-lower halves live in a rotating PSUM tile drained by
                    # the DVE only; v-upper halves live in the persistent
                    # chain drained by the ACT only. (Separate tiles per
                    # engine: the Tile framework chains accessors of a tile,
                    # so sharing one would serialise the two drain engines.)
                    ps_lo = pmain.tile([128, 2, 512], F32, tag="ps")
                    ps_hi = chains[half % 2]
                    for tt in range(2):
                        # 128-row one-hot selector = column t of the identity,
                        # broadcast across 128 weight columns via a stride-0
                        # AP (no 4 MB selector tensor needed). Every main-loop
                        # matmul keeps the same (128,128) tile config, so the
                        # PE array never pays a tile-reconfig stall (which
                        # would poison the HAM activity window and de-rate the
                        # clock to 1.2 GHz).
                        t = t0 + tt
                        sel_ap = idn[:, t : t + 1].copy()
                        part = sel_ap.ap.to_list()[0]
                        sel_ap.ap = _vec_pair([part, [0, 128]])
                        # v-lower half: enc broadcast only (dec added by DVE).
                        nc.tensor.matmul(
                            ps_lo[:, tt, :],
                            lhsT=sel_ap,
                            rhs=enc_hi[:, 0:512],
                            start=True,
                            stop=True,
                            tile_position=(0, 0),
                            skip_group_check=True,
                        )
                        # v-upper half: first visit initialises the chain with
                        # enc(t); later visits accumulate enc(t) - enc(t-4).
                        nc.tensor.matmul(
                            ps_hi[:, tt, :],
                            lhsT=sel_ap,
                            rhs=(enc_hi if init else enc_dif)[:, 512:1024],
                            start=init,
                            stop=not init,
                            tile_position=(0, 0),
                            skip_group_check=True,
                        )
                    if init:
                        # Chain init: accumulate dec once via identity matmuls
                        # (back to back so the PE loads the idn weights once).
                        for tt in range(2):
                            nc.tensor.matmul(
                                ps_hi[:, tt, :],
                                lhsT=idn[:],
                                rhs=dec2[:, 0, 512:1024],
                                start=False,
                                stop=True,
                                tile_position=(0, 0),
                                skip_group_check=True,
                            )
                    # Drain both tiles of the pair in one instruction per
                    # engine: DVE adds dec to the lower halves, ACT copies
                    # the upper halves (dec already in PSUM).
                    nc.vector.tensor_add(
                        out=ob_dve[:, 2 * half : 2 * half + 2, :],
                        in0=ps_lo[:],
                        in1=dec2[:, :, 0:512],
                    )
                    nc.scalar.copy(
                        out=ob_act[:, 2 * half : 2 * half + 2, :],
                        in_=ps_hi[:],
                    )
                nc.sync.dma_start(out=OUT[grp, 0], in_=ob_dve[:])
                nc.sync.dma_start(out=OUT[grp, 1], in_=ob_act[:])
    nc.compile()
    return nc


def _to_bf16(a: np.ndarray) -> np.ndarray:
    # numpy has no bfloat16; round-to-nearest-even to bf16 kept in a uint16
    # view, which is what run_bass_kernel_spmd expects for BF16 tensors.
    try:
        import ml_dtypes

        return a.astype(ml_dtypes.bfloat16)
    except ImportError:
        x = a.astype(np.float32).view(np.uint32)
        rounded = (x + 0x7FFF + ((x >> 16) & 1)) >> 16
        return rounded.astype(np.uint16)


_PROGRAM = None


def _get_program() -> bass.Bass:
    global _PROGRAM
    if _PROGRAM is None:
        _PROGRAM = _build_program()
    return _PROGRAM


def _make_in_maps(inputs):
    enc = np.asarray(inputs["encoder_outputs"], dtype=np.float32)
    dec = np.asarray(inputs["decoder_outputs"], dtype=np.float32)
    W = np.asarray(inputs["W"], dtype=np.float32)
    WT = np.ascontiguousarray(W.T)  # (2D, V)
    IDN = _to_bf16(np.eye(128, dtype=np.float32))
    # DIF[k, t] = 1 if k == t, -1 if k == t - 4: one matmul turns enc_hi
    # into the per-step chain increments enc_hi[t] - enc_hi[t-4].
    dif = np.eye(128, dtype=np.float32)
    for t in range(4, 128):
        dif[t - 4, t] = -1.0
    DIF = _to_bf16(dif)
    in_maps = []
    for c in range(N_CORES):
        b = c // (N_CORES // B)
        t0 = (c % (N_CORES // B)) * T_LOC
        encT = enc[b, t0 : t0 + T_LOC, :].T  # (D, T_LOC)
        decT = dec[b].T  # (D, U)
        pack = np.empty((8, 128, PKW), np.float32)
        for kc in range(4):
            pack[kc, :, :128] = encT[128 * kc : 128 * (kc + 1), :]
            pack[kc, :, 128:] = WT[128 * kc : 128 * (kc + 1), :]
        for kc in range(4, 8):
            pack[kc, :, :128] = decT[128 * (kc - 4) : 128 * (kc - 3), :]
            pack[kc, :, 128:] = WT[128 * kc : 128 * (kc + 1), :]
        in_maps.append({"PACK": _to_bf16(pack), "IDNR": IDN, "DIFR": DIF})
    return in_maps


def _unpermute(slab: np.ndarray) -> np.ndarray:
    # (T_LOC//TG, 2, U, TG, 512) -> (T_LOC, U, V)
    return slab.transpose(0, 3, 2, 1, 4).reshape(T_LOC, U, V)


def _assemble(results) -> np.ndarray:
    out = np.empty((B, T, U, V), np.float32)
    for c in range(N_CORES):
        b = c // (N_CORES // B)
        t0 = (c % (N_CORES // B)) * T_LOC
        out[b, t0 : t0 + T_LOC] = _unpermute(np.asarray(results[c]["out"]))
    return out


def _run(inputs, **spmd_kwargs):
    nc = _get_program()
    in_maps = _make_in_maps(inputs)
    res = run_bass_kernel_spmd(nc, in_maps, core_ids=list(range(N_CORES)), **spmd_kwargs)
    return _assemble(res.results), res


def kernel(**inputs) -> np.ndarray:
    out, _ = _run(inputs)
    return out



# revision 2
# speedup vs baseline: 1.0311x; 1.0311x over previous
"""RNN-T joint network (Conformer transducer) kernel for Trainium2 — v5.

out[b,t,u,v] = enc_proj[b,t,v] + dec_proj[b,u,v], quantized to uint8.

Sharding: tensor-parallel over V. Core c owns v in [128c, 128c+128) and
emits the full (B*T=1024, U=128) grid for its v-slice: 16.78M u8 elements
(16 MB) per core. The fp32 output is dequantized on the host; the 2e-2
max-relative error budget comfortably covers the ~0.004 quantization step.

Per-core structure (v on SBUF partitions), two parallel lanes:
  - DVE lane (blocks 0-7 minus half of 7): four big tensor_adds
    (~2 blocks each, FD up to 16K), in0 = encS col-slice broadcast over u
    (stride-0 free dim), in1 = decS broadcast over bt; f32 operands
    (same 1x DVE speed as 16-bit here, zero operand rounding), u8 out.
    Few big ops because each op->DMA handoff costs a ~2us DVE DRAIN.
  - PE+ACT lane (half of 7 + blocks 8-15): persistent 4-bank PSUM chains
    (two 16-row tiles ping-ponged across the lane's whole bt range).
    Each step accumulates a *projected input difference*
    dif32 = (enc[t]-enc[t-32]) @ W_enc.T via one identity-matmul per
    bank (stride-0 rhs broadcast over u) — half the PE work of plain
    re-adds. The dif path is projected in fp16 (separate fp16 W copy)
    to keep the chain's random-walk drift small. ACT drains FD=2048
    with the quantization affine (scale/bias APs) for free. Chains
    re-seed at the b=0/b=1 boundary where dec changes.
  No GPSIMD lane: the Q7 cores share a physical SBUF port with the DVE;
  a concurrent gpsimd tensor_add slows DVE tensor_tensors ~45% (net loss).

Inputs arrive in two packed DMAs (first one carries W + the first enc
quarter so the pipeline starts ~3us in). float->u8 conversion
truncates-and-wraps, so the +0.5 rounding offset is folded into the
zero-point and the range stays inside [2, 253]. Each block is written by
a 1 MB HWDGE DMA (8 KB/partition descriptors); host dequantizes +
transposes.
"""

import numpy as np

import concourse.bass as bass
import concourse.tile as tile
from concourse import bacc
from concourse import mybir
from concourse.bass_utils import run_bass_kernel_spmd

B, T, U, D, V = 2, 512, 128, 512, 1024
N_CORES = 8
BT = B * T            # 1024 grid rows
NBLK = 16             # bt-blocks per core
BLK = BT // NBLK      # 64 bt-rows per block
NQ = 4
QW = BT // NQ         # 256 bt per quarter
CH = 16               # chain tile rows
DIF = 32              # chain step (2 tiles ping-pong)
A_T0 = 480            # ACT lane covers bt in [480, 1024)

F32 = mybir.dt.float32
F16 = mybir.dt.float16
BF16 = mybir.dt.bfloat16
U8 = mybir.dt.uint8

# packed input 0: [wtc (8x128) | enctq0 (4x256)] bf16 -> 2+2 KB/partition
# packed input 1: [enctq1..3 (12x256) | dect (4x256) | idn (128)] bf16
IN0_W = 8 * 128 + 4 * QW
IN1_W = 12 * QW + 4 * (B * U) + 128
# packed input 2 (fp16): [wtc16 (4x128) | dift (4x512)]
IN2_W = 4 * 128 + 4 * T


def _build_program(s_f: float, zp_f: float) -> bass.Bass:
    nc = bacc.Bacc("TRN2", debug=False, num_devices=N_CORES)

    IN0 = nc.dram_tensor("IN0", [128, IN0_W], BF16, kind="ExternalInput").ap()
    IN1 = nc.dram_tensor("IN1", [128, IN1_W], BF16, kind="ExternalInput").ap()
    IN2 = nc.dram_tensor("IN2", [128, IN2_W], F16, kind="ExternalInput").ap()
    # OUT[blk, v, j, u] u8, bt = blk*64 + j
    OUT = nc.dram_tensor("out", [NBLK, 128, BLK, U], U8, kind="ExternalOutput").ap()

    with tile.TileContext(nc) as tc:
        with (
            tc.tile_pool(name="const", bufs=1) as cpool,
            tc.tile_pool(name="pch", bufs=1, space="PSUM") as pch,
            tc.tile_pool(name="od", bufs=3) as odve,
            tc.tile_pool(name="oa", bufs=3) as oact,
        ):
            in0 = cpool.tile([128, IN0_W], BF16, tag="in0")
            nc.sync.dma_start(out=in0[:], in_=IN0)
            in1 = cpool.tile([128, IN1_W], BF16, tag="in1")
            nc.sync.dma_start(out=in1[:], in_=IN1)
            in2 = cpool.tile([128, IN2_W], F16, tag="in2")
            nc.sync.dma_start(out=in2[:], in_=IN2)

            def wt(k):      # [128, 128] bf16 — W chunk k (k<4 enc, k>=4 dec)
                return in0[:, 128 * k : 128 * (k + 1)]

            def enct(q, dc):  # [128, QW] bf16 — enc.T quarter q, d-chunk dc
                if q == 0:
                    return in0[:, 1024 + QW * dc : 1024 + QW * (dc + 1)]
                off = QW * (4 * (q - 1) + dc)
                return in1[:, off : off + QW]

            def dect(dc):
                off = 12 * QW + (B * U) * dc
                return in1[:, off : off + B * U]

            idn = in1[:, 12 * QW + 4 * (B * U) :]

            def wt16(dc):
                return in2[:, 128 * dc : 128 * (dc + 1)]

            def dift(dc, j0, cols):
                off = 512 + T * dc + j0
                return in2[:, off : off + cols]

            chA = pch.tile([128, CH, 128], F32, tag="chA")
            chB = pch.tile([128, CH, 128], F32, tag="chB")

            def proj(dst_ps, k0, rhs_of_dc):
                for dc in range(4):
                    nc.tensor.matmul(
                        dst_ps,
                        lhsT=wt(k0 + dc),
                        rhs=rhs_of_dc(dc),
                        start=(dc == 0),
                        stop=(dc == 3),
                    )

            encS = [None] * 2
            encB = {}
            difB = cpool.tile([128, T], BF16, tag="difB")
            decS = cpool.tile([128, B * U], F16, tag="decS")
            decB = cpool.tile([128, B * U], BF16, tag="decB")

            def drain_S(dst, ps):
                nc.scalar.activation(
                    out=dst, in_=ps,
                    func=mybir.ActivationFunctionType.Copy,
                    bias=(zp_f + 0.5) / 2.0, scale=s_f,
                )

            # q0 -> chA  (D lane starts ASAP)
            ps = chA[:, 0:2, :]
            proj(ps, 0, lambda dc: enct(0, dc))
            encS[0] = cpool.tile([128, QW], F16, tag="encS0", name="encS0")
            drain_S(encS[0][:], ps)
            # dec -> chA (needed by D lane first op too)
            ps = chA[:, 2:4, :]
            proj(ps, 4, dect)
            drain_S(decS[:], ps)
            nc.scalar.copy(out=decB[:], in_=ps)
            # q1 -> chB
            ps = chB[:, 0:2, :]
            proj(ps, 0, lambda dc: enct(1, dc))
            encS[1] = cpool.tile([128, QW], F16, tag="encS1", name="encS1")
            drain_S(encS[1][:], ps)
            encB[1] = cpool.tile([128, QW], BF16, tag="encB1", name="encB1")
            nc.scalar.copy(out=encB[1][:], in_=ps)
            # q2 -> chB (encB only: chain inits at the b boundary)
            ps = chB[:, 2:4, :]
            proj(ps, 0, lambda dc: enct(2, dc))
            encB[2] = cpool.tile([128, QW], BF16, tag="encB2", name="encB2")
            nc.scalar.copy(out=encB[2][:], in_=ps)
            # dif halves (fp16 path) -> chA / chB
            ps = chA[:, 4:6, :]
            for dc in range(4):
                nc.tensor.matmul(ps, lhsT=wt16(dc), rhs=dift(dc, 0, 256),
                                 start=(dc == 0), stop=(dc == 3))
            nc.scalar.copy(out=difB[:, 0:256], in_=ps)
            ps = chB[:, 4:6, :]
            for dc in range(4):
                nc.tensor.matmul(ps, lhsT=wt16(dc), rhs=dift(dc, 256, 256),
                                 start=(dc == 0), stop=(dc == 3))
            nc.scalar.copy(out=difB[:, 256:512], in_=ps)

            # ---- DVE lane: 4 big ops over blocks [0,2) [2,4) [4,6) [6,7.5) --
            def dve_op(t0, t1):
                rows = t1 - t0
                q = t0 // QW
                assert (t1 - 1) // QW == q
                b = t0 // T
                ot = odve.tile([128, rows, U], U8, tag=f"do{rows}", name=f"do{t0}")
                in_e = encS[q][:, t0 % QW : t0 % QW + rows].unsqueeze(2)
                in_d = decS[:, b * U : (b + 1) * U].unsqueeze(1)
                nc.vector.tensor_add(
                    out=ot[:],
                    in0=in_e.broadcast_to([128, rows, U]),
                    in1=in_d.broadcast_to([128, rows, U]),
                )
                r = 0
                while r < rows:
                    blk = (t0 + r) // BLK
                    j0 = (t0 + r) % BLK
                    n = min(BLK - j0, rows - r)
                    nc.sync.dma_start(
                        out=OUT[blk, :, j0 : j0 + n, :], in_=ot[:, r : r + n, :]
                    )
                    r += n

            # ---- ACT lane: chained PSUM tiles over bt in [480, 1024) ----
            chain = [chA, chB]

            def act_tile(t0, ot, orow):
                k = (t0 - A_T0) // CH
                pm = chain[k % 2]
                b = t0 // T
                init = t0 in (480, 496, 512, 528)
                for bank in range(4):
                    tb = t0 + 4 * bank
                    dst_ps = pm[:, 4 * bank : 4 * bank + 4, :]
                    if init:
                        q = tb // QW
                        rhs_enc = (
                            encB[q][:, tb % QW : tb % QW + 4]
                            .unsqueeze(2).broadcast_to([128, 4, U])
                        )
                        rhs_dec = (
                            decB[:, b * U : (b + 1) * U]
                            .unsqueeze(1).broadcast_to([128, 4, U])
                        )
                        nc.tensor.matmul(dst_ps, lhsT=idn, rhs=rhs_enc,
                                         start=True, stop=False)
                        nc.tensor.matmul(dst_ps, lhsT=idn, rhs=rhs_dec,
                                         start=False, stop=True)
                    else:
                        rhs_dif = (
                            difB[:, tb - 512 : tb - 512 + 4]
                            .unsqueeze(2).broadcast_to([128, 4, U])
                        )
                        nc.tensor.matmul(dst_ps, lhsT=idn, rhs=rhs_dif,
                                         start=False, stop=True,
                                         tile_position=(0, 0),
                                         skip_group_check=True)
                nc.scalar.activation(
                    out=ot[:, orow : orow + CH, :], in_=pm[:],
                    func=mybir.ActivationFunctionType.Copy,
                    bias=zp_f + 0.5, scale=s_f,
                )

            def act_unit(blk, half):
                rows = BLK if half is None else BLK // 2
                t0 = blk * BLK + (0 if half in (None, 0) else BLK // 2)
                dst = (
                    OUT[blk]
                    if half is None
                    else OUT[blk, :, half * rows : half * rows + rows, :]
                )
                ot = oact.tile([128, rows, U], U8, tag=f"ao{rows}", name=f"ao{t0}")
                for seg in range(rows // CH):
                    act_tile(t0 + seg * CH, ot, seg * CH)
                nc.sync.dma_start(out=dst, in_=ot[:])

            # ---- main: interleave lanes (independent engine queues) ----
            dve_ops = [(0, 128), (128, 256), (256, 384), (384, 480)]
            act_units = [(7, 1)] + [(blk, None) for blk in range(8, 16)]
            order = [
                ("D", 0), ("A", 0), ("A", 1), ("D", 1), ("A", 2), ("A", 3),
                ("D", 2), ("A", 4), ("A", 5), ("D", 3), ("A", 6), ("A", 7),
                ("A", 8),
            ]
            for lane, i in order:
                if lane == "D":
                    dve_op(*dve_ops[i])
                else:
                    act_unit(*act_units[i])
    nc.compile()
    return nc


def _to_bf16(a):
    import ml_dtypes

    return np.asarray(a, dtype=np.float32).astype(ml_dtypes.bfloat16)


_PROGRAM = {}


def _get_program(s_f: float, zp_f: float) -> bass.Bass:
    key = (s_f, zp_f)
    if key not in _PROGRAM:
        _PROGRAM[key] = _build_program(s_f, zp_f)
    return _PROGRAM[key]


def _quant_params(enc_flat, dec_flat, W):
    encp = enc_flat @ W[:, :D].T  # (BT, V)
    decp = dec_flat @ W[:, D:].T  # (B*U, V)
    e = encp.reshape(B, T, V)
    d = decp.reshape(B, U, V)
    hi = float((e.max(axis=1) + d.max(axis=1)).max())
    lo = float((e.min(axis=1) + d.min(axis=1)).min())
    s = 248.0 / (hi - lo)
    zq = 3.5 - s * lo
    return s, zq


def _make_in_maps(inputs):
    import ml_dtypes

    enc = np.asarray(inputs["encoder_outputs"], dtype=np.float32)
    dec = np.asarray(inputs["decoder_outputs"], dtype=np.float32)
    W = np.asarray(inputs["W"], dtype=np.float32)
    enc_flat = np.ascontiguousarray(enc.reshape(BT, D))
    dec_flat = np.ascontiguousarray(dec.reshape(B * U, D))
    s, zp = _quant_params(enc_flat, dec_flat, W)

    # enc.T quarters: ET[q, p, dc, j] = enc_flat[q*QW+j, dc*128+p]
    ET = enc_flat.reshape(NQ, QW, 4, 128).transpose(0, 3, 2, 1)
    DT = dec_flat.reshape(B * U, 4, 128).transpose(2, 1, 0)  # [p, dc, bu]
    dif = enc_flat[T:] - enc_flat[T - DIF : BT - DIF]  # rows 512..1023
    DIFT = dif.reshape(T, 4, 128).transpose(2, 1, 0)  # [p, dc, j]
    IDN = np.eye(128, dtype=np.float32)

    in1_common = np.concatenate(
        [ET[1].reshape(128, -1), ET[2].reshape(128, -1), ET[3].reshape(128, -1),
         DT.reshape(128, -1), IDN],
        axis=1,
    )
    IN1 = _to_bf16(np.ascontiguousarray(in1_common))
    in_maps = []
    for c in range(N_CORES):
        wslice = W[128 * c : 128 * (c + 1), :]  # (128v, 1024)
        WTC = wslice.T.reshape(8, 128, 128).transpose(1, 0, 2)  # [p, k, v]
        IN0 = _to_bf16(
            np.ascontiguousarray(
                np.concatenate([WTC.reshape(128, -1), ET[0].reshape(128, -1)], axis=1)
            )
        )
        IN2c = np.ascontiguousarray(
            np.concatenate(
                [WTC[:, 0:4].reshape(128, -1), DIFT.reshape(128, -1)], axis=1
            )
        ).astype(np.float16)
        in_maps.append({"IN0": IN0, "IN1": IN1, "IN2": IN2c})
    return in_maps, s, zp


def _dequant(slab, s, zp):
    x = slab.astype(np.float32)
    x -= zp
    x *= 1.0 / s
    return x.transpose(0, 2, 3, 1).reshape(BT, U, 128)


def _assemble(results, s, zp) -> np.ndarray:
    out = np.empty((BT, U, V), np.float32)
    for c in range(N_CORES):
        out[:, :, 128 * c : 128 * (c + 1)] = _dequant(
            np.asarray(results[c]["out"]), s, zp
        )
    return out.reshape(B, T, U, V)


def _run(inputs, **spmd_kwargs):
    in_maps, s, zp = _make_in_maps(inputs)
    nc = _get_program(s, zp)
    res = run_bass_kernel_spmd(nc, in_maps, core_ids=list(range(N_CORES)), **spmd_kwargs)
    return _assemble(res.results, s, zp), res


def kernel(**inputs) -> np.ndarray:
    out, _ = _run(inputs)
    return out


# revision 4
# speedup vs baseline: 1.0586x; 1.0267x over previous
"""RNN-T joint network (Conformer transducer) kernel for Trainium2 — v5.

out[b,t,u,v] = enc_proj[b,t,v] + dec_proj[b,u,v], quantized to uint8.

Sharding: tensor-parallel over V. Core c owns v in [128c, 128c+128) and
emits the full (B*T=1024, U=128) grid for its v-slice: 16.78M u8 elements
(16 MB) per core. The fp32 output is dequantized on the host; the 2e-2
max-relative error budget comfortably covers the ~0.004 quantization step.

Per-core structure (v on SBUF partitions), two parallel lanes:
  - DVE lane (blocks 0-7 minus half of 7): four big tensor_adds
    (~2 blocks each, FD up to 16K), in0 = encS col-slice broadcast over u
    (stride-0 free dim), in1 = decS broadcast over bt; f32 operands
    (same 1x DVE speed as 16-bit here, zero operand rounding), u8 out.
    Few big ops because each op->DMA handoff costs a ~2us DVE DRAIN.
  - PE+ACT lane (half of 7 + blocks 8-15): persistent 4-bank PSUM chains
    (two 16-row tiles ping-ponged across the lane's whole bt range).
    Each step accumulates a *projected input difference*
    dif32 = (enc[t]-enc[t-32]) @ W_enc.T via one identity-matmul per
    bank (stride-0 rhs broadcast over u) — half the PE work of plain
    re-adds. The dif path is projected in fp16 (separate fp16 W copy)
    to keep the chain's random-walk drift small. ACT drains FD=2048
    with the quantization affine (scale/bias APs) for free. Chains
    re-seed at the b=0/b=1 boundary where dec changes.
  No GPSIMD lane: the Q7 cores share a physical SBUF port with the DVE;
  a concurrent gpsimd tensor_add slows DVE tensor_tensors ~45% (net loss).

Inputs arrive in two packed DMAs (first one carries W + the first enc
quarter so the pipeline starts ~3us in). float->u8 conversion
truncates-and-wraps, so the +0.5 rounding offset is folded into the
zero-point and the range stays inside [2, 253]. Each block is written by
a 1 MB HWDGE DMA (8 KB/partition descriptors); host dequantizes +
transposes.
"""

import numpy as np

import concourse.bass as bass
import concourse.tile as tile
from concourse import bacc
from concourse import mybir
from concourse.bass_utils import run_bass_kernel_spmd

B, T, U, D, V = 2, 512, 128, 512, 1024
N_CORES = 8
BT = B * T            # 1024 grid rows
NBLK = 16             # bt-blocks per core
BLK = BT // NBLK      # 64 bt-rows per block
NQ = 4
QW = BT // NQ         # 256 bt per quarter
CH = 16               # chain tile rows
DIF = 32              # chain step (2 tiles ping-pong)
A_T0 = 544            # ACT lane covers bt in [544, 1024)

F32 = mybir.dt.float32
F16 = mybir.dt.float16
BF16 = mybir.dt.bfloat16
U8 = mybir.dt.uint8

# packed input 0: [wtc (8x128) | enctq0 (4x256)] bf16
# packed input DC: [dect (4x256)] bf16
# packed input 1: [enctq1..3 (12x256) | idn (128)] bf16
IN0_W = 8 * 128 + 4 * QW
INDC_W = 4 * (B * U)
IN1_W = 12 * QW + 128
# packed input 2 (fp16): [wtc16 (4x128) | dift (4x512)]
IN2_W = 4 * 128 + 4 * T


def _build_program(s_f: float, zp_f: float) -> bass.Bass:
    nc = bacc.Bacc("TRN2", debug=False, num_devices=N_CORES)

    IN0 = nc.dram_tensor("IN0", [128, IN0_W], BF16, kind="ExternalInput").ap()
    INDC = nc.dram_tensor("INDC", [128, INDC_W], BF16, kind="ExternalInput").ap()
    IN1 = nc.dram_tensor("IN1", [128, IN1_W], BF16, kind="ExternalInput").ap()
    IN2 = nc.dram_tensor("IN2", [128, IN2_W], F16, kind="ExternalInput").ap()
    # D lane (bt in [0,512)): OUTD[v, u, t]; block 8: OUTD8[v, u, j]
    OUTD = nc.dram_tensor("outd", [128, U, T], U8, kind="ExternalOutput").ap()
    OUTD8 = nc.dram_tensor("outd8", [128, BLK // 2, U], U8, kind="ExternalOutput").ap()
    # A lane (bt in [544,1024)): OUTA[blk-8, v, j, u]; blk-8 rows 0:32 unused
    OUTA = nc.dram_tensor("outa", [8, 128, BLK, U], U8, kind="ExternalOutput").ap()

    with tile.TileContext(nc) as tc:
        with (
            tc.tile_pool(name="const", bufs=1) as cpool,
            tc.tile_pool(name="pch", bufs=1, space="PSUM") as pch,
            tc.tile_pool(name="od", bufs=3) as odve,
            tc.tile_pool(name="oa", bufs=3) as oact,
        ):
            in0 = cpool.tile([128, IN0_W], BF16, tag="in0")
            nc.sync.dma_start(out=in0[:], in_=IN0)
            indc = cpool.tile([128, INDC_W], BF16, tag="indc")
            nc.sync.dma_start(out=indc[:], in_=INDC)
            in1 = cpool.tile([128, IN1_W], BF16, tag="in1")
            nc.sync.dma_start(out=in1[:], in_=IN1)
            in2 = cpool.tile([128, IN2_W], F16, tag="in2")
            nc.sync.dma_start(out=in2[:], in_=IN2)

            def wt(k):      # [128, 128] bf16 — W chunk k (k<4 enc, k>=4 dec)
                return in0[:, 128 * k : 128 * (k + 1)]

            def enct(q, dc):  # [128, QW] bf16 — enc.T quarter q, d-chunk dc
                if q == 0:
                    return in0[:, 1024 + QW * dc : 1024 + QW * (dc + 1)]
                off = QW * (4 * (q - 1) + dc)
                return in1[:, off : off + QW]

            def dect(dc):
                return indc[:, (B * U) * dc : (B * U) * (dc + 1)]

            idn = in1[:, 12 * QW :]

            def wt16(dc):
                return in2[:, 128 * dc : 128 * (dc + 1)]

            def dift(dc, j0, cols):
                off = 512 + T * dc + j0
                return in2[:, off : off + cols]

            chA = pch.tile([128, CH, 128], F32, tag="chA")
            chB = pch.tile([128, CH, 128], F32, tag="chB")

            def proj(dst_ps, k0, rhs_of_dc):
                for dc in range(4):
                    nc.tensor.matmul(
                        dst_ps,
                        lhsT=wt(k0 + dc),
                        rhs=rhs_of_dc(dc),
                        start=(dc == 0),
                        stop=(dc == 3),
                    )

            encB = {}
            difB = cpool.tile([128, T], BF16, tag="difB")
            decS = cpool.tile([128, B * U], F32, tag="decS")
            decS16 = cpool.tile([128, B * U], F16, tag="decS16")
            decB = cpool.tile([128, B * U], BF16, tag="decB")

            def drain_S(dst, ps):
                nc.scalar.activation(
                    out=dst, in_=ps,
                    func=mybir.ActivationFunctionType.Copy,
                    bias=(zp_f + 0.5) / 2.0, scale=s_f,
                )

            encS = cpool.tile([128, 2 * QW], F16, tag="encS", name="encS")
            encS2 = cpool.tile([128, QW], F16, tag="encS2", name="encS2")
            # q0 -> chA  (D lane starts ASAP)
            ps = chA[:, 0:2, :]
            proj(ps, 0, lambda dc: enct(0, dc))
            drain_S(encS[:, 0:QW], ps)
            # dec -> chA (needed by D lane first op too)
            ps = chA[:, 2:4, :]
            proj(ps, 4, dect)
            drain_S(decS[:], ps)
            drain_S(decS16[:], ps)
            nc.scalar.copy(out=decB[:], in_=ps)
            # q1 -> chB
            ps = chB[:, 0:2, :]
            proj(ps, 0, lambda dc: enct(1, dc))
            drain_S(encS[:, QW : 2 * QW], ps)
            # q2 -> chB (S for block-8 strip, B for chain inits)
            ps = chB[:, 2:4, :]
            proj(ps, 0, lambda dc: enct(2, dc))
            drain_S(encS2[:], ps)
            encB[2] = cpool.tile([128, QW], BF16, tag="encB2", name="encB2")
            nc.scalar.copy(out=encB[2][:], in_=ps)
            # dif halves (fp16 path) -> chA / chB
            ps = chA[:, 4:6, :]
            for dc in range(4):
                nc.tensor.matmul(ps, lhsT=wt16(dc), rhs=dift(dc, 0, 256),
                                 start=(dc == 0), stop=(dc == 3))
            nc.scalar.copy(out=difB[:, 0:256], in_=ps)
            ps = chB[:, 4:6, :]
            for dc in range(4):
                nc.tensor.matmul(ps, lhsT=wt16(dc), rhs=dift(dc, 256, 256),
                                 start=(dc == 0), stop=(dc == 3))
            nc.scalar.copy(out=difB[:, 256:512], in_=ps)

            # ---- DVE lane: per-u tensor_scalar planes (2x DVE mode) ----
            def dve_chunk(u0, nu):
                # u-plane chunk over the full b=0 range t in [0, 512)
                ot = odve.tile([128, nu, T], U8, tag="dc", name=f"dc{u0}")
                for ui in range(nu):
                    nc.vector.tensor_scalar(
                        ot[:, ui, :], encS[:, 0:T],
                        decS[:, u0 + ui : u0 + ui + 1], None,
                        mybir.AluOpType.add,
                    )
                nc.sync.dma_start(out=OUTD[:, u0 : u0 + nu, :], in_=ot[:])

            def dve_blk8():
                # block-8 first half via one tensor_add: t in [512, 544), b=1
                rows = BLK // 2
                ot = odve.tile([128, rows, U], U8, tag="d8", name="d8")
                in_e = encS2[:, 0:rows].unsqueeze(2).broadcast_to([128, rows, U])
                in_d = decS16[:, U : 2 * U].unsqueeze(1).broadcast_to([128, rows, U])
                nc.vector.tensor_add(out=ot[:], in0=in_e, in1=in_d)
                nc.sync.dma_start(out=OUTD8, in_=ot[:])

            # ---- ACT lane: chained PSUM tiles over bt in [480, 1024) ----
            chain = [chA, chB]

            def act_tile(t0, ot, orow):
                k = (t0 - A_T0) // CH
                pm = chain[k % 2]
                b = t0 // T
                init = t0 in (544, 560)
                for bank in range(4):
                    tb = t0 + 4 * bank
                    dst_ps = pm[:, 4 * bank : 4 * bank + 4, :]
                    if init:
                        q = tb // QW
                        rhs_enc = (
                            encB[q][:, tb % QW : tb % QW + 4]
                            .unsqueeze(2).broadcast_to([128, 4, U])
                        )
                        rhs_dec = (
                            decB[:, b * U : (b + 1) * U]
                            .unsqueeze(1).broadcast_to([128, 4, U])
                        )
                        nc.tensor.matmul(dst_ps, lhsT=idn, rhs=rhs_enc,
                                         start=True, stop=False)
                        nc.tensor.matmul(dst_ps, lhsT=idn, rhs=rhs_dec,
                                         start=False, stop=True)
                    else:
                        rhs_dif = (
                            difB[:, tb - 512 : tb - 512 + 4]
                            .unsqueeze(2).broadcast_to([128, 4, U])
                        )
                        nc.tensor.matmul(dst_ps, lhsT=idn, rhs=rhs_dif,
                                         start=False, stop=True,
                                         tile_position=(0, 0),
                                         skip_group_check=True)
                nc.scalar.activation(
                    out=ot[:, orow : orow + CH, :], in_=pm[:],
                    func=mybir.ActivationFunctionType.Copy,
                    bias=zp_f + 0.5, scale=s_f,
                )

            def act_unit(blk, row0, rows):
                t0 = blk * BLK + row0
                dst = (
                    OUTA[blk - 8]
                    if rows == BLK
                    else OUTA[blk - 8, :, row0 : row0 + rows, :]
                )
                ot = oact.tile([128, rows, U], U8, tag=f"ao{rows}", name=f"ao{t0}")
                for seg in range(rows // CH):
                    act_tile(t0 + seg * CH, ot, seg * CH)
                nc.sync.dma_start(out=dst, in_=ot[:])

            # ---- main: interleave lanes (independent engine queues) ----
            act_units = (
                [(8, 32, 32)] + [(blk, 0, BLK) for blk in range(9, 15)]
                + [(15, 0, 32), (15, 32, 16), (15, 48, 16)]
            )
            ai = 0
            for g in range(8):            # u chunks 0..127
                dve_chunk(16 * g, 16)
                if g % 2 == 1 and ai < len(act_units):
                    act_unit(*act_units[ai]); ai += 1
                if g == 3:
                    dve_blk8()
            while ai < len(act_units):
                act_unit(*act_units[ai]); ai += 1
    nc.compile()
    return nc


def _to_bf16(a):
    import ml_dtypes

    return np.asarray(a, dtype=np.float32).astype(ml_dtypes.bfloat16)


_PROGRAM = {}


def _get_program(s_f: float, zp_f: float) -> bass.Bass:
    key = (s_f, zp_f)
    if key not in _PROGRAM:
        _PROGRAM[key] = _build_program(s_f, zp_f)
    return _PROGRAM[key]


def _quant_params(enc_flat, dec_flat, W):
    encp = enc_flat @ W[:, :D].T  # (BT, V)
    decp = dec_flat @ W[:, D:].T  # (B*U, V)
    e = encp.reshape(B, T, V)
    d = decp.reshape(B, U, V)
    hi = float((e.max(axis=1) + d.max(axis=1)).max())
    lo = float((e.min(axis=1) + d.min(axis=1)).min())
    s = 248.0 / (hi - lo)
    zq = 3.5 - s * lo
    return s, zq


def _make_in_maps(inputs):
    import ml_dtypes

    enc = np.asarray(inputs["encoder_outputs"], dtype=np.float32)
    dec = np.asarray(inputs["decoder_outputs"], dtype=np.float32)
    W = np.asarray(inputs["W"], dtype=np.float32)
    enc_flat = np.ascontiguousarray(enc.reshape(BT, D))
    dec_flat = np.ascontiguousarray(dec.reshape(B * U, D))
    s, zp = _quant_params(enc_flat, dec_flat, W)

    # enc.T quarters: ET[q, p, dc, j] = enc_flat[q*QW+j, dc*128+p]
    ET = enc_flat.reshape(NQ, QW, 4, 128).transpose(0, 3, 2, 1)
    DT = dec_flat.reshape(B * U, 4, 128).transpose(2, 1, 0)  # [p, dc, bu]
    dif = enc_flat[T:] - enc_flat[T - DIF : BT - DIF]  # rows 512..1023
    DIFT = dif.reshape(T, 4, 128).transpose(2, 1, 0)  # [p, dc, j]
    IDN = np.eye(128, dtype=np.float32)

    DTB = _to_bf16(np.ascontiguousarray(DT.reshape(128, -1)))
    in1_common = np.concatenate(
        [ET[1].reshape(128, -1), ET[2].reshape(128, -1), ET[3].reshape(128, -1),
         IDN],
        axis=1,
    )
    IN1 = _to_bf16(np.ascontiguousarray(in1_common))
    in_maps = []
    for c in range(N_CORES):
        wslice = W[128 * c : 128 * (c + 1), :]  # (128v, 1024)
        WTC = wslice.T.reshape(8, 128, 128).transpose(1, 0, 2)  # [p, k, v]
        IN0 = _to_bf16(
            np.ascontiguousarray(
                np.concatenate([WTC.reshape(128, -1), ET[0].reshape(128, -1)], axis=1)
            )
        )
        IN2c = np.ascontiguousarray(
            np.concatenate(
                [WTC[:, 0:4].reshape(128, -1), DIFT.reshape(128, -1)], axis=1
            )
        ).astype(np.float16)
        in_maps.append({"IN0": IN0, "INDC": DTB, "IN1": IN1, "IN2": IN2c})
    return in_maps, s, zp


def _assemble_core(outd, outd8, outa, s, zp) -> np.ndarray:
    """(outd [v,u,t<512], outd8 [v,u,64], outa [7,v,j,u]) -> (BT, U, 128v)."""
    inv = 1.0 / s
    full = np.empty((BT, U, 128), np.float32)
    full[0:T] = ((outd.astype(np.float32) - zp) * inv).transpose(2, 1, 0)
    full[T:] = (
        ((outa.astype(np.float32) - zp) * inv).transpose(0, 2, 3, 1).reshape(-1, U, 128)
    )
    full[T : T + BLK // 2] = (
        (outd8.astype(np.float32) - zp) * inv
    ).transpose(1, 2, 0)
    return full


def _assemble(results, s, zp) -> np.ndarray:
    out = np.empty((BT, U, V), np.float32)
    for c in range(N_CORES):
        r = results[c]
        out[:, :, 128 * c : 128 * (c + 1)] = _assemble_core(
            np.asarray(r["outd"]), np.asarray(r["outd8"]), np.asarray(r["outa"]), s, zp
        )
    return out.reshape(B, T, U, V)


def _run(inputs, **spmd_kwargs):
    in_maps, s, zp = _make_in_maps(inputs)
    nc = _get_program(s, zp)
    res = run_bass_kernel_spmd(nc, in_maps, core_ids=list(range(N_CORES)), **spmd_kwargs)
    return _assemble(res.results, s, zp), res


def kernel(**inputs) -> np.ndarray:
    out, _ = _run(inputs)
    return out
